# revision 32
# baseline (speedup 1.0000x reference)
"""Cross-attention Trainium2 kernel (8 NeuronCores, data-parallel).

Problem: B=4, C=64, H=64, W=64.
  q = conv1x1(v1, wq, bq); k = conv1x1(v2, wk, bk); v = conv1x1(v2, wv, bv)
  tokens n = (c, h) pairs (N = C*H = 4096), feature dim = W = 64
  out = softmax(q @ k^T) @ v

Sharding: core i handles batch b = i//2 and the q-token half h in
[32*(i%2), 32*(i%2+1)).  Every core needs the full v2[b] (k/v side) but only
its h-slice of v1[b] (q side).  No collectives.

Per-core algorithm (v2: warm-clock + bf16 scores + split exp):
  - PE warmup burst at kernel start trips the HAM clock gate to 8/8
    (2.4 GHz) before the projections; the whole kernel then runs warm.
  - everything on the matmul operand path is bf16 (scores matmuls are
    1-pass instead of fp32's 2-pass); accumulation stays fp32 in PSUM.
  - biases are folded into the PSUM->SBUF copies (ACT Identity+bias /
    DVE tensor_scalar add) instead of ones-row augmentation.
  - scores computed TRANSPOSED: sT[j, i] = k_j . q_i with k-tokens j on
    partitions; after exp the tile is exactly the stationary-operand layout
    the P@V matmul needs.  Two k-token blocks run concurrently in the PE
    via row groups (contraction is only W=64).
  - exp is split across engines: even p-blocks on ScalarE (table exp ->
    bf16), odd p-blocks on VectorE as a one-pass Schraudolph fast-exp:
    int16 = round(s * 128*log2e + (127*128 - 5.504)), whose bit pattern
    IS bf16(exp(s)) to ~3%; softmax normalization cancels most of it.
  - no max subtraction (|s| <= ~74 here; exp fits bf16's range); softmax
    denominator via a ones-column appended to V.
"""

import numpy as np

B, C, H, W = 4, 64, 64, 64
HH = H // 2            # h-rows per core (q-token half)
NQ = C * HH            # q tokens per core = 2048
NK = C * H             # k tokens = 4096
JB = NK // 128         # 32 j-blocks of 128 k-tokens
NP = JB // 2           # 16 row-packed j-block pairs
IP = 512               # i-span per pass (4 passes)
NCORES = 8

# Schraudolph fast-exp constants (bf16 bit pattern via int16):
# i16 = round(s * 128*log2e + 127*128 - 0.043*128)
A_SCH = 128.0 * 1.4426950408889634
B_SCH = 127.0 * 128.0 - 0.0430 * 128.0

USE_SCH = True       # odd p-blocks use DVE Schraudolph fast-exp
USE_BIAS_FOLD = True # biases via copy-stage (vs skipped entirely)
USE_WARMUP = True    # PE warmup burst

_CACHE = {}


def _build_nc():
    from contextlib import ExitStack

    import concourse.bass as bass
    import concourse.tile as tile
    from concourse import bacc, mybir
    from concourse.bass import ts
    from concourse.masks import make_identity

    F32 = mybir.dt.float32
    BF16 = mybir.dt.bfloat16
    F32R = mybir.dt.float32r
    I16 = mybir.dt.int16
    AF = mybir.ActivationFunctionType
    ALU = mybir.AluOpType

    nc = bacc.Bacc(trn_type="TRN2", target_bir_lowering=False)

    x1 = nc.declare_dram_parameter("x1", [C, HH * W], F32, False)
    x2 = nc.declare_dram_parameter("x2", [C, H * W], F32, False)
    wq_d = nc.declare_dram_parameter("wq", [C, C], F32, False)
    wk_d = nc.declare_dram_parameter("wk", [C, C], F32, False)
    wv_d = nc.declare_dram_parameter("wv", [C, C], F32, False)
    bq_d = nc.declare_dram_parameter("bq", [C, 1], F32, False)
    bk_d = nc.declare_dram_parameter("bk", [C, 1], F32, False)
    bv_d = nc.declare_dram_parameter("bv", [C, 1], F32, False)
    out_d = nc.declare_dram_parameter("out", [C, HH, W], F32, True)

    with ExitStack() as ctx:
        tc = ctx.enter_context(tile.TileContext(nc))
        cp = ctx.enter_context(tc.tile_pool(name="const", bufs=1))

        # inputs start moving immediately
        x1_st = cp.tile([C, HH * W], F32)
        x2_st = cp.tile([C, H * W], F32)
        w_sb = {}
        for name, wd in (("q", wq_d), ("k", wk_d), ("v", wv_d)):
            t = cp.tile([C, C], F32, tag=f"w_{name}")
            nc.scalar.dma_start(t[:, :], wd[:, :])
            w_sb[name] = t
        b_sb = {}
        for name, bd in (("q", bq_d), ("k", bk_d), ("v", bv_d)):
            t = cp.tile([C, 1], F32, tag=f"b_{name}")
            nc.scalar.dma_start(t[:, :], bd[:, :])
            b_sb[name] = t
        nc.scalar.dma_start(x1_st[:, 0:1024], x1[:, 0:1024])
        nc.sync.dma_start(x1_st[:, 1024:2048], x1[:, 1024:2048])
        nc.scalar.dma_start(x2_st[:, :], x2[:, :])

        ident = cp.tile([128, 128], F32)
        make_identity(nc, ident[:, :])

        # prewarm the exp table set while input DMAs run
        warm = cp.tile([128, 2], F32)
        nc.vector.memset(warm[:, :], 0.0)
        nc.scalar.activation(warm[:, 0:1], warm[:, 1:2], AF.Exp)

        # ---- PE warmup: ~6us of contiguous matmul busy trips the HAM
        # clock gate to 8/8 (2.4 GHz) before any real PE work ----
        if USE_WARMUP:
            ws = cp.tile([128, 256], BF16)
            nc.gpsimd.memset(ws[:, :], 0.0)
            with tc.tile_pool(name="wp", bufs=1, space="PSUM") as wp:
                wps = wp.tile([128, 256], F32, tag="warm")
                for _ in range(34):
                    nc.tensor.matmul(
                        wps[:, :], lhsT=ws[:, 0:128], rhs=ws[:, :],
                        start=True, stop=True,
                    )

        # bf16 operand staging for the projections
        x1_sb = cp.tile([C, HH * W], F32R)
        x2_sb = cp.tile([C, H * W], F32R)
        for c in range(HH * W // 1024):
            if c % 2:
                nc.scalar.activation(
                    x1_sb[:, ts(c, 1024)], x1_st[:, ts(c, 1024)], AF.Copy
                )
            else:
                nc.vector.tensor_copy(x1_sb[:, ts(c, 1024)], x1_st[:, ts(c, 1024)])
        for c in range(H * W // 1024):
            if c % 2:
                nc.scalar.activation(
                    x2_sb[:, ts(c, 1024)], x2_st[:, ts(c, 1024)], AF.Copy
                )
            else:
                nc.vector.tensor_copy(x2_sb[:, ts(c, 1024)], x2_st[:, ts(c, 1024)])

        # wT (bf16): rows = c_in, cols = c_out
        wT = {}
        with tc.tile_pool(name="pp0", bufs=2, space="PSUM") as pp0:
            for name in ("q", "k", "v"):
                t = cp.tile([C, C], F32R, tag=f"wT_{name}")
                ps = pp0.tile([C, C], F32, tag="wT_ps")
                nc.tensor.transpose(ps[:, :], w_sb[name][:, :], ident[0:C, 0:C])
                nc.vector.tensor_copy(t[:, :], ps[:, :])
                wT[name] = t

        # ---- projections (channel-major, bias folded) + transposes ----
        Q_cm = cp.tile([C, HH * W], F32)
        K_cm = cp.tile([C, H * W], F32)
        # qT2: (w, i) duplicated on both partition halves (rhs of scores)
        # kT2: (w, j) even j-blocks on partitions 0-63, odd on 64-127 (lhsT)
        qT2 = cp.tile([128, NQ], BF16)
        kT2 = cp.tile([128, NK // 2], BF16)

        # vf_aug (128, JB, 65) bf16: partition p of block jb = v-token
        # (c = p%64, h = 2*jb + p//64); col 64 = 1.0 (denominator trick)
        vf = cp.tile([128, JB, 65], BF16)
        nc.gpsimd.memset(vf[:, :, 64:65], 1.0)

        _cp_n = [0]

        def psum_copy(dst, src, bias=None, allow_act=True):
            if not USE_BIAS_FOLD:
                bias = None
            if allow_act and _cp_n[0] % 2 == 0:
                if bias is None:
                    nc.scalar.activation(dst, src, AF.Copy)
                else:
                    nc.scalar.activation(dst, src, AF.Identity, bias=bias[:, 0:1])
            else:
                if bias is None:
                    nc.vector.tensor_copy(dst, src)
                else:
                    nc.vector.tensor_scalar(
                        dst, src, bias[:, 0:1], None, ALU.add
                    )
            _cp_n[0] += 1

        with tc.tile_pool(name="pp1", bufs=3, space="PSUM") as pp1:
            def project(dst, wTt, x_sb, ch, bias, allow_act=False):
                ps = pp1.tile([C, 1024], F32, tag="setup")
                for c2 in range(2):
                    nc.tensor.matmul(
                        ps[:, ts(c2, 512)],
                        lhsT=wTt[:, :],
                        rhs=x_sb[:, ch * 1024 + c2 * 512 :][:, 0:512],
                        start=True, stop=True,
                    )
                psum_copy(dst[:, ts(ch, 1024)], ps[:, :], bias, allow_act)

            def project_v(ch, allow_act=False):
                # V: psum -> vf directly (bf16 cast + (h2,h1,w) rearrange)
                ps = pp1.tile([C, 1024], F32, tag="setup")
                for c2 in range(2):
                    nc.tensor.matmul(
                        ps[:, ts(c2, 512)],
                        lhsT=wT["v"][:, :],
                        rhs=x2_sb[:, ch * 1024 + c2 * 512 :][:, 0:512],
                        start=True, stop=True,
                    )
                pv = ps[:, :].rearrange("p (h2 h1 w) -> p h1 h2 w", h1=2, w=W)
                for h1 in range(2):
                    dst = vf[64 * h1 : 64 * (h1 + 1), ts(ch, 8), 0:W]
                    psum_copy(dst, pv[:, h1, :, :], b_sb["v"], allow_act)

            def q_transpose(grp, allow_act=False):
                ps = pp1.tile([64, 1024], F32, tag="setup")
                for hh in range(16):
                    h = grp * 16 + hh
                    nc.tensor.transpose(
                        ps[:, ts(hh, 64)], Q_cm[:, ts(h, 64)], ident[0:C, 0:C]
                    )
                psum_copy(qT2[0:64, ts(grp, 1024)], ps[:, :], None, allow_act)
                psum_copy(qT2[64:128, ts(grp, 1024)], ps[:, :], None, allow_act)

            def k_transpose(grp, allow_act=False):
                ps = pp1.tile([64, 1024], F32, tag="setup")
                for hh in range(16):
                    h = grp * 16 + hh
                    nc.tensor.transpose(
                        ps[:, ts(hh, 64)], K_cm[:, ts(h, 64)], ident[0:C, 0:C]
                    )
                pv = ps[:, :].rearrange("p (b two c) -> p b two c", two=2, c=128)
                for half in range(2):
                    dst = kT2[64 * half : 64 * half + 64, ts(grp, 512)].rearrange(
                        "p (b c) -> p b c", c=128
                    )
                    psum_copy(dst, pv[:, :, half, :], None, allow_act)

            # staggered emission: chunk g's transposes are emitted after
            # chunk g+1's projections so the in-order PE queue never waits
            # on the psum->sbuf copy of the chunk it just produced; pass 0
            # only needs qT2 group 0, so q_transpose(1) goes last
            for ch in range(HH * W // 1024):
                project(Q_cm, wT["q"], x1_sb, ch, b_sb["q"], allow_act=True)
            project(K_cm, wT["k"], x2_sb, 0, b_sb["k"], allow_act=True)
            project_v(0, allow_act=True)
            q_transpose(0, allow_act=True)
            project(K_cm, wT["k"], x2_sb, 1, b_sb["k"], allow_act=True)
            project_v(1, allow_act=True)
            k_transpose(0, allow_act=True)
            for ch in range(2, H * W // 1024):
                project(K_cm, wT["k"], x2_sb, ch, b_sb["k"], allow_act=True)
                project_v(ch, allow_act=True)
                k_transpose(ch - 1, allow_act=True)
            q_transpose(1, allow_act=True)
            k_transpose(H // 16 - 1, allow_act=True)

        # ---- main attention loop: 4 passes over i, row-packed j pairs ----
        # One PSUM tile per pair holds block A (cols 0-511) and block B
        # (cols 512-1023) at the same i-window: the two scores matmuls are
        # adjacent and overlap in the PE array (row groups 0-1 vs 2-3), and
        # a single FD=1024 exp covers both blocks.  exp alternates between
        # ScalarE (table exp) and VectorE (Schraudolph int16 fast-exp).
        outT_sb = cp.tile([C + 1, NQ], F32)
        with (
            tc.tile_pool(name="outp", bufs=1, space="PSUM") as op_pool,
            tc.tile_pool(name="sp", bufs=3, space="PSUM") as sp,
            tc.tile_pool(name="ppool", bufs=4) as p_pool,
            tc.tile_pool(name="tp2", bufs=1, space="PSUM") as tp2,
            tc.tile_pool(name="opool", bufs=4) as o_pool,
            tc.tile_pool(name="rpool", bufs=4) as r_pool,
        ):
            pending_tails = []

            def emit_tail(ihh):
                for tt in range(IP // 128):
                    t = ihh * (IP // 128) + tt
                    ps = tp2.tile([128, C + 1], F32, tag="ot")
                    nc.tensor.transpose(
                        ps[:, :], outT_sb[:, ts(t, 128)],
                        ident[0 : C + 1, 0 : C + 1],
                    )
                    rec = r_pool.tile([128, 1], F32, tag="rec")
                    nc.vector.reciprocal(rec[:, :], ps[:, C : C + 1])
                    ot = o_pool.tile([128, C], F32, tag="o")
                    nc.vector.tensor_scalar_mul(ot[:, :], ps[:, 0:C], rec[:, 0:1])
                    dest = out_d[:, 2 * t : 2 * t + 2, :].rearrange("o h w -> h o w")
                    nc.sync.dma_start(dest, ot[:, :])

            for ih in range(NQ // IP):
                i0 = ih * IP
                outT_ps = op_pool.tile([C + 1, IP], F32, tag="outT")
                pts2 = {}
                for p in range(NP + 2):
                    if p < NP:
                        sps = sp.tile([128, 2 * IP], F32, tag="scores")
                        for blk in range(2):
                            half = 64 * blk
                            nc.tensor.matmul(
                                sps[:, ts(blk, IP)],
                                lhsT=kT2[half : half + 64, ts(p, 128)],
                                rhs=qT2[half : half + 64, i0 : i0 + IP],
                                start=True, stop=True,
                            )
                        if p % 2 == 0 or not USE_SCH:
                            pt = p_pool.tile([128, 2 * IP], BF16, tag="p")
                            nc.scalar.activation(pt[:, :], sps[:, :], AF.Exp)
                        else:
                            pt = p_pool.tile([128, 2 * IP], BF16, tag="p")
                            nc.vector.tensor_scalar(
                                pt[:, :].bitcast(I16), sps[:, :], A_SCH, B_SCH,
                                ALU.mult, ALU.add,
                            )
                        pts2[p] = pt
                    if p == 3 and pending_tails:
                        emit_tail(pending_tails.pop(0))
                    if p >= 2:
                        q = p - 2
                        ptq = pts2.pop(q)
                        for blk in range(2):
                            jb = 2 * q + blk
                            nc.tensor.matmul(
                                outT_ps[:, :],
                                lhsT=vf[:, jb, :],
                                rhs=ptq[:, ts(blk, IP)],
                                start=(q == 0 and blk == 0),
                                stop=(q == NP - 1 and blk == 1),
                            )
                # drain this pass's accumulator to SBUF; the normalize +
                # store blocks are DEFERRED into the next pass so their PE
                # transposes (which wait on the drain) don't head-of-line
                # block the next pass's score matmuls
                dst = outT_sb[:, i0 : i0 + IP]
                if ih % 2 == 0:
                    nc.scalar.activation(dst, outT_ps[:, :], AF.Copy)
                else:
                    nc.vector.tensor_copy(dst, outT_ps[:, :])
                pending_tails.append(ih)
                if ih == NQ // IP - 1:
                    while pending_tails:
                        emit_tail(pending_tails.pop(0))

    nc.compile()
    return nc


def _get_nc():
    if "nc" not in _CACHE:
        _CACHE["nc"] = _build_nc()
    return _CACHE["nc"]


def _in_maps(v1, v2, wq, bq, wk, bk, wv, bv):
    maps = []
    for core in range(NCORES):
        b, half = divmod(core, 2)
        maps.append({
            "x1": np.ascontiguousarray(
                v1[b, :, half * HH : (half + 1) * HH, :], dtype=np.float32
            ).reshape(C, HH * W),
            "x2": np.ascontiguousarray(v2[b], dtype=np.float32).reshape(C, H * W),
            "wq": np.ascontiguousarray(wq, dtype=np.float32),
            "wk": np.ascontiguousarray(wk, dtype=np.float32),
            "wv": np.ascontiguousarray(wv, dtype=np.float32),
            "bq": np.ascontiguousarray(bq, dtype=np.float32).reshape(C, 1),
            "bk": np.ascontiguousarray(bk, dtype=np.float32).reshape(C, 1),
            "bv": np.ascontiguousarray(bv, dtype=np.float32).reshape(C, 1),
        })
    return maps


def _gather(results, v1):
    out = np.zeros((B, C, H, W), dtype=np.float32)
    for core in range(NCORES):
        b, half = divmod(core, 2)
        out[b, :, half * HH : (half + 1) * HH, :] = results[core]["out"]
    return out


def _run(trace=False, **inputs):
    from concourse.bass_utils import run_bass_kernel_spmd

    nc = _get_nc()
    maps = _in_maps(**inputs)
    res = run_bass_kernel_spmd(
        nc, maps, core_ids=list(range(NCORES)), trace=trace
    )
    return _gather(res.results, inputs["v1"]), res


def kernel(**inputs):
    out, _ = _run(trace=False, **inputs)
    return out
